# revision 18
# baseline (speedup 1.0000x reference)
"""Trainium2 8-core kernel for biased-attention with sigmoid gating.

Reference computation (per batch b):
  q = heads(q_x @ Wq) * C**-0.5 ; k = heads(kv_x @ Wk) ; v = heads(kv_x @ Wv)
  a = softmax(q k^T + bias1 + bias2, axis=-1)
  o = (a @ v) gated by sigmoid(q_x @ Wg + bg), then @ Wo + bo

Shapes: B=2, Q=K=2048, CQ=CK=CV=256, H=8, C=32, CO=256.

Sharding: 8 cores = 2 batches x 4 query-quarters (512 rows each). Each core
computes all 8 heads for its rows; no cross-core communication is needed.
The dominant cost is streaming the two [B,H,Q,K] f32 bias tensors (67 MB per
core); bias2 is DMA'd onto bias1's SBUF tile with an inline CCE add so the
sum costs no engine time.
"""

import numpy as np

B, Q, K, CQ, H, C, CO = 2, 2048, 2048, 256, 8, 32, 256
HC = H * C  # 256
QS = Q // 4  # 512 query rows per core
N_CORES = 8
SCALE = float(C) ** -0.5

_CACHED = {}


def _build():
    import concourse.bass as bass
    import concourse.mybir as mybir
    import concourse.tile as tile
    from concourse import bacc
    from concourse.masks import make_identity

    f32 = mybir.dt.float32
    bf16 = mybir.dt.bfloat16
    AF = mybir.ActivationFunctionType
    ALU = mybir.AluOpType

    nc = bacc.Bacc(None, target_bir_lowering=False)

    qx = nc.declare_dram_parameter("qx", [QS, CQ], f32, isOutput=False)
    kvx = nc.declare_dram_parameter("kvx", [K, CQ], f32, isOutput=False)
    # biases arrive host-transposed: [H, K, QS] (k-major), so score tiles can
    # be produced directly in the transposed [k, q] orientation
    b1 = nc.declare_dram_parameter("b1", [H, K, QS], f32, isOutput=False)
    b2 = nc.declare_dram_parameter("b2", [H, K, QS], f32, isOutput=False)
    Wq = nc.declare_dram_parameter("Wq", [CQ, HC], f32, isOutput=False)
    Wk = nc.declare_dram_parameter("Wk", [CQ, HC], f32, isOutput=False)
    Wv = nc.declare_dram_parameter("Wv", [CQ, HC], f32, isOutput=False)
    Wg = nc.declare_dram_parameter("Wg", [CQ, HC], f32, isOutput=False)
    bg = nc.declare_dram_parameter("bg", [HC], f32, isOutput=False)
    Wo = nc.declare_dram_parameter("Wo", [HC, CO], f32, isOutput=False)
    bo = nc.declare_dram_parameter("bo", [CO], f32, isOutput=False)
    out = nc.declare_dram_parameter("out", [QS, CO], f32, isOutput=True)

    with tile.TileContext(nc) as tc:
        with (
            tc.tile_pool(name="singles", bufs=1) as singles,
            tc.tile_pool(name="stage", bufs=3) as stage,
            tc.tile_pool(name="bias", bufs=3) as biasp,
            tc.tile_pool(name="work", bufs=3) as work,
            tc.tile_pool(name="ework", bufs=3) as ework,
            tc.tile_pool(name="ps", bufs=1, space="PSUM") as psp,
        ):
            ident = singles.tile([128, 128], bf16)
            make_identity(nc, ident)
            identf = singles.tile([128, 128], f32, tag="identf")
            make_identity(nc, identf)

            # ---- weights: load f32, cast to bf16, split into 2 row-chunks ----
            wbf = {}
            for name, w in (("Wq", Wq), ("Wk", Wk), ("Wv", Wv), ("Wg", Wg), ("Wo", Wo)):
                wtile = singles.tile([128, 2, 256], bf16, tag=f"w_{name}")
                for ck in range(2):
                    wf = stage.tile([128, 256], f32, tag="wstage")
                    nc.sync.dma_start(out=wf, in_=w[ck * 128:(ck + 1) * 128, :])
                    nc.vector.tensor_copy(wtile[:, ck, :], wf)
                wbf[name] = wtile

            # broadcast bg / bo across partitions
            bg_bc = singles.tile([128, HC], f32, tag="bg")
            nc.sync.dma_start(out=bg_bc, in_=bg[:].partition_broadcast(128))
            bo_bc = singles.tile([128, CO], f32, tag="bo")
            nc.sync.dma_start(out=bo_bc, in_=bo[:].partition_broadcast(128))

            # ---- transpose inputs: qxT [256ck, 512q], kvxT [256ck, 2048k] (bf16) ----
            qxT = singles.tile([128, 2, QS], bf16, tag="qxT")
            for rt in range(QS // 128):  # 4 row tiles
                xf = stage.tile([128, CQ], f32, tag="xstage")
                nc.sync.dma_start(out=xf, in_=qx[rt * 128:(rt + 1) * 128, :])
                xb = stage.tile([128, CQ], bf16, tag="xbf")
                nc.vector.tensor_copy(xb, xf)
                for ck in range(2):
                    tp = psp.tile([128, 128], bf16, tag="et_ps", bufs=2)
                    nc.tensor.transpose(tp, xb[:, ck * 128:(ck + 1) * 128], ident)
                    nc.any.tensor_copy(qxT[:, ck, rt * 128:(rt + 1) * 128], tp)
            kvxT = singles.tile([128, 2, K], bf16, tag="kvxT")
            for rt in range(K // 128):  # 16 row tiles
                xf = stage.tile([128, CQ], f32, tag="xstage")
                nc.sync.dma_start(out=xf, in_=kvx[rt * 128:(rt + 1) * 128, :])
                xb = stage.tile([128, CQ], bf16, tag="xbf")
                nc.vector.tensor_copy(xb, xf)
                for ck in range(2):
                    tp = psp.tile([128, 128], bf16, tag="et_ps", bufs=2)
                    nc.tensor.transpose(tp, xb[:, ck * 128:(ck + 1) * 128], ident)
                    nc.any.tensor_copy(kvxT[:, ck, rt * 128:(rt + 1) * 128], tp)

            # ---- projections (bf16 matmuls, f32 psum) ----
            # Per-head transposed projections, heads stacked on the free dim
            # (PE operands must start at base partition 0/32/64, so a
            # 4-heads-per-128-partitions packing is not usable as lhsT).
            # QT [32c, 8h, 512q] scaled by C^-0.5 ; KT [32c, 8h, 2048k]
            QT = singles.tile([32, H, QS], bf16, tag="QT")
            for h in range(H):
                ps = psp.tile([128, QS, 1], f32, tag="scores", bufs=3)
                for ck in range(2):
                    nc.tensor.matmul(
                        ps[:32, :, 0],
                        wbf["Wq"][:, ck, h * 32:(h + 1) * 32],
                        qxT[:, ck, :],
                        start=(ck == 0),
                        stop=(ck == 1),
                    )
                nc.vector.tensor_scalar_mul(QT[:, h, :], ps[:32, :, 0], SCALE)
            KT = singles.tile([32, H, K], bf16, tag="KT")
            for h in range(H):
                for kc in range(4):
                    ps = psp.tile([128, 512, 1], f32, tag="scores", bufs=3)
                    for ck in range(2):
                        nc.tensor.matmul(
                            ps[:32, :, 0],
                            wbf["Wk"][:, ck, h * 32:(h + 1) * 32],
                            kvxT[:, ck, kc * 512:(kc + 1) * 512],
                            start=(ck == 0),
                            stop=(ck == 1),
                        )
                    nc.any.tensor_copy(KT[:, h, kc * 512:(kc + 1) * 512], ps[:32, :, 0])

            # V natural [128kr, 16kt, 8h*33] bf16; per head 32 V columns plus
            # an all-ones column so the PV matmul emits softmax denominators
            # for free in output column 32.
            Vn = singles.tile([128, K // 128, H * 33], bf16, tag="Vn")
            nc.vector.memset(Vn, 1.0)
            for kt in range(K // 128):
                ps = psp.tile([128, HC, 1], f32, tag="scores", bufs=3)
                for ck in range(2):
                    nc.tensor.matmul(
                        ps[:, :, 0],
                        kvxT[:, ck, kt * 128:(kt + 1) * 128],
                        wbf["Wv"][:, ck, :],
                        start=(ck == 0),
                        stop=(ck == 1),
                    )
                for h in range(H):
                    nc.any.tensor_copy(
                        Vn[:, kt, h * 33:h * 33 + 32], ps[:, h * 32:(h + 1) * 32, 0]
                    )

            # G natural [128q, 4qt, 256hc] f32 = sigmoid(qx @ Wg + bg)
            Gn = singles.tile([128, 4, HC], f32, tag="Gn")
            for qt in range(4):
                ps = psp.tile([128, HC, 1], f32, tag="scores", bufs=3)
                for ck in range(2):
                    nc.tensor.matmul(
                        ps[:, :, 0],
                        qxT[:, ck, qt * 128:(qt + 1) * 128],
                        wbf["Wg"][:, ck, :],
                        start=(ck == 0),
                        stop=(ck == 1),
                    )
                gt = stage.tile([128, HC], f32, tag="gtmp")
                nc.vector.tensor_add(gt, ps[:, :, 0], bg_bc)
                nc.scalar.activation(Gn[:, qt, :], gt, AF.Sigmoid)

            # ---- main attention loops (transposed orientation) ----
            # Per head: stream host-transposed bias tiles B^T [128k, 512q],
            # sum them on GpSimd (bf16), add into the QK^T PSUM bank via an
            # identity matmul, exp on ScalarE straight out of PSUM, and feed
            # E^T to the PV matmul as the moving operand. Softmax denominators
            # come from V_aug's ones column; a tiny [33,128] back-transpose
            # restores natural orientation for the per-row normalization.
            O_all = singles.tile([128, 4, HC], f32, tag="O_all")
            KTILES = K // 128  # 16
            for h in range(H):
                hcol = h * 32
                o_ps = psp.tile([33, QS, 1], f32, tag="o_acc", bufs=2)
                for half in range(2):
                    # one 2 MB DMA per half-head: 8 k-tiles packed on the free
                    # dim ("(a p) q -> p a q"), per-partition runs stay 2 KB
                    # contiguous rows
                    B1t = biasp.tile([128, 8, QS], f32, tag="b1", bufs=2)
                    B2t = biasp.tile([128, 8, QS], f32, tag="b2", bufs=2)
                    Bsum = biasp.tile([128, 8, QS], bf16, tag="bsum", bufs=2)
                    rows = slice(half * 1024, (half + 1) * 1024)
                    nc.sync.dma_start(
                        out=B1t, in_=b1[h, rows, :].rearrange("(a p) q -> p a q", p=128)
                    )
                    nc.scalar.dma_start(
                        out=B2t, in_=b2[h, rows, :].rearrange("(a p) q -> p a q", p=128)
                    )
                    for sub in range(8):
                        kt = half * 8 + sub
                        nc.gpsimd.tensor_tensor(
                            Bsum[:, sub, :], B1t[:, sub, :], B2t[:, sub, :], ALU.add
                        )
                        s_ps = psp.tile([128, QS, 1], f32, tag="scores", bufs=3)
                        nc.tensor.matmul(
                            s_ps[:, :, 0],
                            KT[:, h, kt * 128:(kt + 1) * 128],
                            QT[:, h, :],
                            start=True,
                            stop=False,
                        )
                        nc.tensor.matmul(
                            s_ps[:, :, 0], ident, Bsum[:, sub, :], start=False, stop=True
                        )
                        et_sb = ework.tile([128, QS], bf16, tag="et")
                        nc.scalar.activation(et_sb, s_ps[:, :, 0], AF.Exp)
                        nc.tensor.matmul(
                            o_ps[:, :, 0],
                            Vn[:, kt, hcol + h:hcol + h + 33],
                            et_sb,
                            start=(kt == 0),
                            stop=(kt == KTILES - 1),
                        )
                oT_sb = work.tile([33, QS], f32, tag="oT")
                nc.vector.tensor_copy(oT_sb, o_ps[:, :, 0])
                for qt in range(4):
                    on_ps = psp.tile([128, C + 1, 1], f32, tag="onat", bufs=1)
                    nc.tensor.transpose(
                        on_ps[:, :, 0],
                        oT_sb[:, qt * 128:(qt + 1) * 128],
                        identf[:33, :33],
                    )
                    rinv = work.tile([128, 1], f32, tag="rinv")
                    nc.vector.reciprocal(rinv, on_ps[:, C:C + 1, 0])
                    nc.vector.tensor_scalar_mul(
                        O_all[:, qt, hcol:hcol + 32], on_ps[:, :C, 0], rinv
                    )

            # ---- gating + output projection ----
            for qt in range(4):
                og = stage.tile([128, HC], bf16, tag="og")
                nc.vector.tensor_mul(og, O_all[:, qt, :], Gn[:, qt, :])
                ogt_ps = psp.tile([128, 2, 128], bf16, tag="et_ps", bufs=2)
                for hcc in range(2):
                    nc.tensor.transpose(
                        ogt_ps[:, hcc, :], og[:, hcc * 128:(hcc + 1) * 128], ident
                    )
                ogt = stage.tile([128, 2, 128], bf16, tag="ogt")
                nc.any.tensor_copy(ogt, ogt_ps)
                f_ps = psp.tile([128, CO, 1], f32, tag="scores", bufs=3)
                for hcc in range(2):
                    nc.tensor.matmul(
                        f_ps[:, :, 0],
                        ogt[:, hcc, :],
                        wbf["Wo"][:, hcc, :],
                        start=(hcc == 0),
                        stop=(hcc == 1),
                    )
                o_sb = stage.tile([128, CO], f32, tag="o_out")
                nc.vector.tensor_add(o_sb, f_ps[:, :, 0], bo_bc)
                nc.sync.dma_start(out=out[qt * 128:(qt + 1) * 128, :], in_=o_sb)

    nc.compile()
    return nc


def _get_nc():
    if "nc" not in _CACHED:
        _CACHED["nc"] = _build()
    return _CACHED["nc"]


def kernel(**inputs):
    from concourse.bass_utils import run_bass_kernel_spmd

    nc = _get_nc()
    inp = {k: np.asarray(v, dtype=np.float32) for k, v in inputs.items()}
    in_maps = []
    for c in range(N_CORES):
        b, qi = c // 4, c % 4
        q0 = qi * QS
        in_maps.append({
            "qx": np.ascontiguousarray(inp["q_x"][b, q0:q0 + QS, :]),
            "kvx": np.ascontiguousarray(inp["kv_x"][b]),
            "b1": np.ascontiguousarray(
                inp["bias1"][b, :, q0:q0 + QS, :].transpose(0, 2, 1)
            ),
            "b2": np.ascontiguousarray(
                inp["bias2"][b, :, q0:q0 + QS, :].transpose(0, 2, 1)
            ),
            "Wq": inp["Wq"], "Wk": inp["Wk"], "Wv": inp["Wv"], "Wg": inp["Wg"],
            "bg": inp["bg"], "Wo": inp["Wo"], "bo": inp["bo"],
        })
    res = run_bass_kernel_spmd(nc, in_maps, core_ids=list(range(N_CORES)))
    outa = np.empty((B, Q, CO), np.float32)
    for c in range(N_CORES):
        b, qi = c // 4, c % 4
        outa[b, qi * QS:(qi + 1) * QS, :] = res.results[c]["out"]
    return outa


# revision 21
# speedup vs baseline: 1.1651x; 1.1651x over previous
"""Trainium2 8-core kernel for biased-attention with sigmoid gating.

Reference computation (per batch b):
  q = heads(q_x @ Wq) * C**-0.5 ; k = heads(kv_x @ Wk) ; v = heads(kv_x @ Wv)
  a = softmax(q k^T + bias1 + bias2, axis=-1)
  o = (a @ v) gated by sigmoid(q_x @ Wg + bg), then @ Wo + bo

Shapes: B=2, Q=K=2048, CQ=CK=CV=256, H=8, C=32, CO=256.

Sharding: 8 cores = 2 batches x 4 query-quarters (512 rows each). Each core
computes all 8 heads for its rows; no cross-core communication is needed.
The dominant cost is streaming the two [B,H,Q,K] f32 bias tensors (67 MB per
core); bias2 is DMA'd onto bias1's SBUF tile with an inline CCE add so the
sum costs no engine time.
"""

import numpy as np

B, Q, K, CQ, H, C, CO = 2, 2048, 2048, 256, 8, 32, 256
HC = H * C  # 256
QS = Q // 4  # 512 query rows per core
N_CORES = 8
SCALE = float(C) ** -0.5

_CACHED = {}


def _build():
    import concourse.bass as bass
    import concourse.mybir as mybir
    import concourse.tile as tile
    from concourse import bacc
    from concourse.masks import make_identity

    f32 = mybir.dt.float32
    bf16 = mybir.dt.bfloat16
    AF = mybir.ActivationFunctionType
    ALU = mybir.AluOpType

    nc = bacc.Bacc(None, target_bir_lowering=False)

    qx = nc.declare_dram_parameter("qx", [QS, CQ], f32, isOutput=False)
    kvx = nc.declare_dram_parameter("kvx", [K, CQ], f32, isOutput=False)
    # biases arrive host-transposed: [H, K, QS] (k-major), so score tiles can
    # be produced directly in the transposed [k, q] orientation
    b1 = nc.declare_dram_parameter("b1", [H, K, QS], f32, isOutput=False)
    b2 = nc.declare_dram_parameter("b2", [H, K, QS], f32, isOutput=False)
    Wq = nc.declare_dram_parameter("Wq", [CQ, HC], f32, isOutput=False)
    Wk = nc.declare_dram_parameter("Wk", [CQ, HC], f32, isOutput=False)
    Wv = nc.declare_dram_parameter("Wv", [CQ, HC], f32, isOutput=False)
    Wg = nc.declare_dram_parameter("Wg", [CQ, HC], f32, isOutput=False)
    bg = nc.declare_dram_parameter("bg", [HC], f32, isOutput=False)
    Wo = nc.declare_dram_parameter("Wo", [HC, CO], f32, isOutput=False)
    bo = nc.declare_dram_parameter("bo", [CO], f32, isOutput=False)
    out = nc.declare_dram_parameter("out", [QS, CO], f32, isOutput=True)

    with tile.TileContext(nc) as tc:
        with (
            tc.tile_pool(name="singles", bufs=1) as singles,
            tc.tile_pool(name="stage", bufs=3) as stage,
            tc.tile_pool(name="bias", bufs=3) as biasp,
            tc.tile_pool(name="work", bufs=3) as work,
            tc.tile_pool(name="ework", bufs=3) as ework,
            tc.tile_pool(name="ps", bufs=1, space="PSUM") as psp,
        ):
            ident = singles.tile([128, 128], bf16)
            make_identity(nc, ident)
            identf = singles.tile([128, 128], f32, tag="identf")
            make_identity(nc, identf)

            # ---- weights: load f32, cast to bf16, split into 2 row-chunks ----
            wbf = {}
            for name, w in (("Wq", Wq), ("Wk", Wk), ("Wv", Wv), ("Wg", Wg), ("Wo", Wo)):
                wtile = singles.tile([128, 2, 256], bf16, tag=f"w_{name}")
                for ck in range(2):
                    wf = stage.tile([128, 256], f32, tag="wstage")
                    nc.sync.dma_start(out=wf, in_=w[ck * 128:(ck + 1) * 128, :])
                    nc.vector.tensor_copy(wtile[:, ck, :], wf)
                wbf[name] = wtile

            # broadcast bg / bo across partitions
            bg_bc = singles.tile([128, HC], f32, tag="bg")
            nc.sync.dma_start(out=bg_bc, in_=bg[:].partition_broadcast(128))
            bo_bc = singles.tile([128, CO], f32, tag="bo")
            nc.sync.dma_start(out=bo_bc, in_=bo[:].partition_broadcast(128))

            # ---- transpose inputs: qxT [256ck, 512q], kvxT [256ck, 2048k] (bf16) ----
            qxT = singles.tile([128, 2, QS], bf16, tag="qxT")
            for rt in range(QS // 128):  # 4 row tiles
                xf = stage.tile([128, CQ], f32, tag="xstage")
                nc.sync.dma_start(out=xf, in_=qx[rt * 128:(rt + 1) * 128, :])
                xb = stage.tile([128, CQ], bf16, tag="xbf")
                nc.vector.tensor_copy(xb, xf)
                for ck in range(2):
                    tp = psp.tile([128, 128], bf16, tag="et_ps", bufs=2)
                    nc.tensor.transpose(tp, xb[:, ck * 128:(ck + 1) * 128], ident)
                    nc.any.tensor_copy(qxT[:, ck, rt * 128:(rt + 1) * 128], tp)
            kvxT = singles.tile([128, 2, K], bf16, tag="kvxT")
            for rt in range(K // 128):  # 16 row tiles
                xf = stage.tile([128, CQ], f32, tag="xstage")
                nc.sync.dma_start(out=xf, in_=kvx[rt * 128:(rt + 1) * 128, :])
                xb = stage.tile([128, CQ], bf16, tag="xbf")
                nc.vector.tensor_copy(xb, xf)
                for ck in range(2):
                    tp = psp.tile([128, 128], bf16, tag="et_ps", bufs=2)
                    nc.tensor.transpose(tp, xb[:, ck * 128:(ck + 1) * 128], ident)
                    nc.any.tensor_copy(kvxT[:, ck, rt * 128:(rt + 1) * 128], tp)

            # ---- projections (bf16 matmuls, f32 psum) ----
            # Per-head transposed projections, heads stacked on the free dim
            # (PE operands must start at base partition 0/32/64, so a
            # 4-heads-per-128-partitions packing is not usable as lhsT).
            # QT [32c, 8h, 512q] scaled by C^-0.5 ; KT [32c, 8h, 2048k]
            QT = singles.tile([32, H, QS], bf16, tag="QT")
            for h in range(H):
                ps = psp.tile([128, QS, 1], f32, tag="scores", bufs=3)
                for ck in range(2):
                    nc.tensor.matmul(
                        ps[:32, :, 0],
                        wbf["Wq"][:, ck, h * 32:(h + 1) * 32],
                        qxT[:, ck, :],
                        start=(ck == 0),
                        stop=(ck == 1),
                    )
                nc.vector.tensor_scalar_mul(QT[:, h, :], ps[:32, :, 0], SCALE)
            KT = singles.tile([32, H, K], bf16, tag="KT")
            for h in range(H):
                for kc in range(4):
                    ps = psp.tile([128, 512, 1], f32, tag="scores", bufs=3)
                    for ck in range(2):
                        nc.tensor.matmul(
                            ps[:32, :, 0],
                            wbf["Wk"][:, ck, h * 32:(h + 1) * 32],
                            kvxT[:, ck, kc * 512:(kc + 1) * 512],
                            start=(ck == 0),
                            stop=(ck == 1),
                        )
                    nc.any.tensor_copy(KT[:, h, kc * 512:(kc + 1) * 512], ps[:32, :, 0])

            # V natural [128kr, 16kt, 8h*33] bf16; per head 32 V columns plus
            # an all-ones column so the PV matmul emits softmax denominators
            # for free in output column 32.
            Vn = singles.tile([128, K // 128, H * 33], bf16, tag="Vn")
            nc.vector.memset(Vn, 1.0)
            for kt in range(K // 128):
                ps = psp.tile([128, HC, 1], f32, tag="scores", bufs=3)
                for ck in range(2):
                    nc.tensor.matmul(
                        ps[:, :, 0],
                        kvxT[:, ck, kt * 128:(kt + 1) * 128],
                        wbf["Wv"][:, ck, :],
                        start=(ck == 0),
                        stop=(ck == 1),
                    )
                for h in range(H):
                    nc.any.tensor_copy(
                        Vn[:, kt, h * 33:h * 33 + 32], ps[:, h * 32:(h + 1) * 32, 0]
                    )

            # G natural [128q, 4qt, 256hc] f32 = sigmoid(qx @ Wg + bg)
            Gn = singles.tile([128, 4, HC], f32, tag="Gn")
            for qt in range(4):
                ps = psp.tile([128, HC, 1], f32, tag="scores", bufs=3)
                for ck in range(2):
                    nc.tensor.matmul(
                        ps[:, :, 0],
                        qxT[:, ck, qt * 128:(qt + 1) * 128],
                        wbf["Wg"][:, ck, :],
                        start=(ck == 0),
                        stop=(ck == 1),
                    )
                gt = stage.tile([128, HC], f32, tag="gtmp")
                nc.vector.tensor_add(gt, ps[:, :, 0], bg_bc)
                nc.scalar.activation(Gn[:, qt, :], gt, AF.Sigmoid)

            # ---- main attention loops (transposed orientation) ----
            # Per head: stream host-transposed bias tiles B^T [128k, 512q],
            # sum them on GpSimd (bf16), add into the QK^T PSUM bank via an
            # identity matmul, exp on ScalarE straight out of PSUM, and feed
            # E^T to the PV matmul as the moving operand. Softmax denominators
            # come from V_aug's ones column; a tiny [33,128] back-transpose
            # restores natural orientation for the per-row normalization.
            O_all = singles.tile([128, 4, HC], f32, tag="O_all")
            KTILES = K // 128  # 16
            for h in range(H):
                hcol = h * 32
                o_ps = psp.tile([33, QS, 1], f32, tag="o_acc", bufs=2)
                for half in range(2):
                    # one 2 MB DMA per half-head: 8 k-tiles packed on the free
                    # dim ("(a p) q -> p a q"), per-partition runs stay 2 KB
                    # contiguous rows
                    B1t = biasp.tile([128, 8, QS], f32, tag="b1", bufs=2)
                    B2t = biasp.tile([128, 8, QS], f32, tag="b2", bufs=2)
                    Bsum = biasp.tile([128, 8, QS], bf16, tag="bsum", bufs=2)
                    rows = slice(half * 1024, (half + 1) * 1024)
                    nc.sync.dma_start(
                        out=B1t, in_=b1[h, rows, :].rearrange("(a p) q -> p a q", p=128)
                    )
                    nc.sync.dma_start(
                        out=B2t, in_=b2[h, rows, :].rearrange("(a p) q -> p a q", p=128)
                    )
                    for sub in range(8):
                        kt = half * 8 + sub
                        nc.gpsimd.tensor_tensor(
                            Bsum[:, sub, :], B1t[:, sub, :], B2t[:, sub, :], ALU.add
                        )
                        s_ps = psp.tile([128, QS, 1], f32, tag="scores", bufs=3)
                        nc.tensor.matmul(
                            s_ps[:, :, 0],
                            KT[:, h, kt * 128:(kt + 1) * 128],
                            QT[:, h, :],
                            start=True,
                            stop=False,
                        )
                        nc.tensor.matmul(
                            s_ps[:, :, 0], ident, Bsum[:, sub, :], start=False, stop=True
                        )
                        et_sb = ework.tile([128, QS], bf16, tag="et")
                        nc.scalar.activation(et_sb, s_ps[:, :, 0], AF.Exp)
                        nc.tensor.matmul(
                            o_ps[:, :, 0],
                            Vn[:, kt, hcol + h:hcol + h + 33],
                            et_sb,
                            start=(kt == 0),
                            stop=(kt == KTILES - 1),
                        )
                oT_sb = work.tile([33, QS], f32, tag="oT")
                nc.vector.tensor_copy(oT_sb, o_ps[:, :, 0])
                for qt in range(4):
                    on_ps = psp.tile([128, C + 1, 1], f32, tag="onat", bufs=1)
                    nc.tensor.transpose(
                        on_ps[:, :, 0],
                        oT_sb[:, qt * 128:(qt + 1) * 128],
                        identf[:33, :33],
                    )
                    rinv = work.tile([128, 1], f32, tag="rinv")
                    nc.vector.reciprocal(rinv, on_ps[:, C:C + 1, 0])
                    nc.vector.tensor_scalar_mul(
                        O_all[:, qt, hcol:hcol + 32], on_ps[:, :C, 0], rinv
                    )

            # ---- gating + output projection ----
            for qt in range(4):
                og = stage.tile([128, HC], bf16, tag="og")
                nc.vector.tensor_mul(og, O_all[:, qt, :], Gn[:, qt, :])
                ogt_ps = psp.tile([128, 2, 128], bf16, tag="et_ps", bufs=2)
                for hcc in range(2):
                    nc.tensor.transpose(
                        ogt_ps[:, hcc, :], og[:, hcc * 128:(hcc + 1) * 128], ident
                    )
                ogt = stage.tile([128, 2, 128], bf16, tag="ogt")
                nc.any.tensor_copy(ogt, ogt_ps)
                f_ps = psp.tile([128, CO, 1], f32, tag="scores", bufs=3)
                for hcc in range(2):
                    nc.tensor.matmul(
                        f_ps[:, :, 0],
                        ogt[:, hcc, :],
                        wbf["Wo"][:, hcc, :],
                        start=(hcc == 0),
                        stop=(hcc == 1),
                    )
                o_sb = stage.tile([128, CO], f32, tag="o_out")
                nc.vector.tensor_add(o_sb, f_ps[:, :, 0], bo_bc)
                nc.sync.dma_start(out=out[qt * 128:(qt + 1) * 128, :], in_=o_sb)

    nc.compile()
    return nc


def _get_nc():
    if "nc" not in _CACHED:
        _CACHED["nc"] = _build()
    return _CACHED["nc"]


def kernel(**inputs):
    from concourse.bass_utils import run_bass_kernel_spmd

    nc = _get_nc()
    inp = {k: np.asarray(v, dtype=np.float32) for k, v in inputs.items()}
    in_maps = []
    for c in range(N_CORES):
        b, qi = c // 4, c % 4
        q0 = qi * QS
        in_maps.append({
            "qx": np.ascontiguousarray(inp["q_x"][b, q0:q0 + QS, :]),
            "kvx": np.ascontiguousarray(inp["kv_x"][b]),
            "b1": np.ascontiguousarray(
                inp["bias1"][b, :, q0:q0 + QS, :].transpose(0, 2, 1)
            ),
            "b2": np.ascontiguousarray(
                inp["bias2"][b, :, q0:q0 + QS, :].transpose(0, 2, 1)
            ),
            "Wq": inp["Wq"], "Wk": inp["Wk"], "Wv": inp["Wv"], "Wg": inp["Wg"],
            "bg": inp["bg"], "Wo": inp["Wo"], "bo": inp["bo"],
        })
    res = run_bass_kernel_spmd(nc, in_maps, core_ids=list(range(N_CORES)))
    outa = np.empty((B, Q, CO), np.float32)
    for c in range(N_CORES):
        b, qi = c // 4, c % 4
        outa[b, qi * QS:(qi + 1) * QS, :] = res.results[c]["out"]
    return outa
